# revision 1
# baseline (speedup 1.0000x reference)
"""Trainium2 Bass kernel for nn_AxisSimplestSpline (PE-accumulated clamp basis).

Math (per batch b, axis a):
  f = A^T raw; g = (f - mins_a)/dx_a in [0,17)

est_a(g) = Y0_a + sum_{k=0..16} s_{a,k} * clamp01(g_a - k),  g = (f-mins)/dx
out[c]   = [sum_a pinv[a,c] Y0_a]          (folded into final ACT-copy bias)
         + sum_k matmul(Wk_fp16, C_k_fp16) (accumulated in output PSUM, fp32)

fp16 features are exact where it matters: C in [0,1] (err <= 2^-12), and
values >= 1 clamp exactly.  Knot terms + output projection fused into 17
fp16 matmuls at 1 cycle/row, accumulated in the output PSUM (start/stop
flags; DVE-produced knots emitted first so the in-order PE never waits on
the slower ACT stream).  Engine split: ACT 11 relus (k=1..11); DVE: g,
boundary knots k=0/16 as single ops (exact by the g-range guarantee),
4 dual-op knots, 15 min-ops at 4x mode, and the output-PSUM drain.
Input projection: host-split fp16 hi/lo raw, 2 matmuls (Ah*h, then
[Al;Ah] against stacked [hi;lo] — error ~2^-22).  Measured: 888 us/core,
rel err 4.4e-4; engines converged (PE ~990 busy, ACT ~894, DVE ~854).
"""

import sys

sys.path.insert(0, "/opt/trn_rl_repo")

import numpy as np

import concourse.bacc as bacc
import concourse.mybir as mybir
import concourse.tile as tile
from concourse.bass_utils import run_bass_kernel_spmd

F32 = mybir.dt.float32
F16 = mybir.dt.float16
EPS = 1e-4
B, C, H, W = 8, 3, 1024, 1024
HW = H * W
NA, K = 8, 16
NK = K + 1
J = 16
NJ = HW // J
FREE = 1024
NSUP = NJ // FREE

ACT_SET = set(range(1, 12))  # ACT relu -> fp16, DVE min @4x; k=0,16 are single-op

_NC_CACHE = {}


def _build_nc():
    nc = bacc.Bacc(None, target_bir_lowering=False, debug=False)
    rawh_t = nc.dram_tensor("rawh", [C, HW], F16, kind="ExternalInput")
    rawl_t = nc.dram_tensor("rawl", [C, HW], F16, kind="ExternalInput")
    # par cols: 0:17 act bias (-mins/dx - k), 17 inv_dx, 18 neg mins/dx
    par_t = nc.dram_tensor("par", [128, 19], F32, kind="ExternalInput")
    wfh_t = nc.dram_tensor("wfh", [C * J, 128], F16, kind="ExternalInput")
    wf2_t = nc.dram_tensor("wf2", [2 * C * J, 128], F16, kind="ExternalInput")
    wks_t = nc.dram_tensor("wks", [128, NK * C * J], F16, kind="ExternalInput")
    bout_t = nc.dram_tensor("bout", [C * J, 1], F32, kind="ExternalInput")
    out_t = nc.dram_tensor("out", [C, HW], F32, kind="ExternalOutput")

    Relu = mybir.ActivationFunctionType.Relu
    Ident = mybir.ActivationFunctionType.Identity
    mult = mybir.AluOpType.mult
    add = mybir.AluOpType.add
    mn = mybir.AluOpType.min
    mx = mybir.AluOpType.max
    sub = mybir.AluOpType.subtract

    with tile.TileContext(nc) as tc:
        with (
            tc.tile_pool(name="const", bufs=1) as cpool,
            tc.tile_pool(name="io", bufs=4) as iopool,
            tc.tile_pool(name="gg", bufs=3) as gpool,
            tc.tile_pool(name="rr", bufs=10) as rpool,
            tc.tile_pool(name="cc", bufs=16) as ccpool,
            tc.tile_pool(name="ob", bufs=3) as obpool,
            tc.tile_pool(name="pf", bufs=2, space="PSUM") as pfpool,
            tc.tile_pool(name="po", bufs=2, space="PSUM") as popool,
        ):
            pT = cpool.tile([128, 19], F32)
            nc.sync.dma_start(out=pT[:], in_=par_t[:])
            wfh = cpool.tile([C * J, 128], F16)
            nc.sync.dma_start(out=wfh[:], in_=wfh_t[:])
            wf2 = cpool.tile([2 * C * J, 128], F16)
            nc.sync.dma_start(out=wf2[:], in_=wf2_t[:])
            wks = cpool.tile([128, NK * C * J], F16)
            nc.sync.dma_start(out=wks[:], in_=wks_t[:])
            bout = cpool.tile([C * J, 1], F32)
            nc.sync.dma_start(out=bout[:], in_=bout_t[:])

            rawh_v = rawh_t.ap().rearrange("c (j n) -> (c j) n", j=J)
            rawl_v = rawl_t.ap().rearrange("c (j n) -> (c j) n", j=J)
            out_v = out_t.ap().rearrange("c (j n) -> (c j) n", j=J)
            NCH = FREE // 512

            for s in range(NSUP):
                n0 = s * FREE
                # stacked rhs: partitions 0:48 = raw_hi, 48:96 = raw_lo
                rhs2 = iopool.tile([2 * C * J, FREE], F16, tag="rhs2")
                nc.sync.dma_start(out=rhs2[: C * J], in_=rawh_v[:, n0 : n0 + FREE])
                nc.sync.dma_start(out=rhs2[C * J :], in_=rawl_v[:, n0 : n0 + FREE])

                # f = (Ah+Al)(h+l) ~= Ah*h + [Ah*l + Al*h]  (error ~2^-22)
                fps = pfpool.tile([128, FREE], F32, tag="fps")
                for h in range(NCH):
                    sl = slice(h * 512, (h + 1) * 512)
                    nc.tensor.matmul(fps[:, sl], wfh[:], rhs2[: C * J, sl], start=True, stop=False)
                    nc.tensor.matmul(fps[:, sl], wf2[:], rhs2[:, sl], start=False, stop=True)

                # g = f*inv_dx - mins*inv_dx (fp32, for the DVE-set knots)
                g = gpool.tile([128, FREE], F32, tag="g")
                nc.vector.tensor_scalar(
                    out=g[:],
                    in0=fps[:],
                    scalar1=pT[:, 17:18],
                    scalar2=pT[:, 18:19],
                    op0=mult,
                    op1=add,
                )

                ops = popool.tile([C * J, FREE], F32, tag="ops")
                korder = [0, 16, 12, 13, 14, 15] + list(range(1, 12))
                for ki, k in enumerate(korder):
                    Ck = ccpool.tile([128, FREE], F16, tag="C")
                    if k == 0:
                        # g >= 0 (and a rounding -eps reproduces the
                        # reference's linear extrapolation exactly)
                        nc.vector.tensor_scalar(
                            out=Ck[:], in0=g[:], scalar1=1.0, scalar2=None, op0=mn
                        )
                    elif k == NK - 1:
                        # g < 17 so relu(g-16) < 1: no upper clamp needed
                        nc.vector.tensor_scalar(
                            out=Ck[:],
                            in0=g[:],
                            scalar1=float(k),
                            scalar2=0.0,
                            op0=sub,
                            op1=mx,
                        )
                    else:
                        Rk = rpool.tile([128, FREE], F16, tag="R")
                        if k in ACT_SET:
                            nc.scalar.activation(
                                Rk[:],
                                fps[:],
                                Relu,
                                bias=pT[:, k : k + 1],
                                scale=pT[:, 17:18],
                            )
                        else:
                            nc.vector.tensor_scalar(
                                out=Rk[:],
                                in0=g[:],
                                scalar1=float(k),
                                scalar2=0.0,
                                op0=sub,
                                op1=mx,
                            )
                        nc.vector.tensor_scalar(
                            out=Ck[:], in0=Rk[:], scalar1=1.0, scalar2=None, op0=mn
                        )
                    wk = wks[:, k * C * J : (k + 1) * C * J]
                    for h in range(NCH):
                        nc.tensor.matmul(
                            ops[:, h * 512 : (h + 1) * 512],
                            wk,
                            Ck[:, h * 512 : (h + 1) * 512],
                            start=(ki == 0),
                            stop=(ki == NK - 1),
                        )

                ob = obpool.tile([C * J, FREE], F32, tag="ob")
                nc.vector.tensor_scalar(
                    out=ob[:], in0=ops[:], scalar1=1.0, scalar2=bout[:, 0:1],
                    op0=mult, op1=add,
                )
                nc.sync.dma_start(out=out_v[:, n0 : n0 + FREE], in_=ob[:])
    nc.compile()
    return nc


def _host_params(raw, ys, A):
    in_maps = []
    for b in range(B):
        Ab = A[b].astype(np.float32)
        mins = np.minimum(Ab, 0).sum(axis=0)
        maxs = np.maximum(Ab, 0).sum(axis=0)
        pinv = np.linalg.pinv(Ab).astype(np.float32)  # [8, 3]
        span = (maxs + np.float32(EPS) - mins).astype(np.float32)
        t = np.linspace(0.0, 1.0, K + 2, dtype=np.float32)
        xs = t[None, :] * span[:, None] + mins[:, None]
        dx = (xs[:, 1] - xs[:, 0]).astype(np.float32)
        Y = np.concatenate(
            [mins[:, None], ys[b].astype(np.float32), maxs[:, None]], axis=1
        )  # [8, 18]
        sg = np.diff(Y, axis=1).astype(np.float32)  # [8, 17]
        inv_dx = (np.float32(1.0) / dx).astype(np.float32)

        par = np.zeros((128, 19), np.float32)
        rep = lambda x: np.repeat(x, J, axis=0)
        ks = np.arange(NK, dtype=np.float32)
        par[:, 0:NK] = rep((-(mins * inv_dx))[:, None] - ks[None, :])
        par[:, 17] = np.repeat(inv_dx, J)
        par[:, 18] = np.repeat(-(mins * inv_dx), J)

        wf = np.zeros((C * J, 128), np.float32)
        for j in range(J):
            for c in range(C):
                for a in range(NA):
                    wf[c * J + j, a * J + j] = Ab[c, a]
        wfh = wf.astype(np.float16)
        wfl = (wf - wfh.astype(np.float32)).astype(np.float16)
        wf2 = np.concatenate([wfl, wfh], axis=0)  # rows 0:48 hit hi, 48:96 hit lo

        wks = np.zeros((128, NK * C * J), np.float16)
        for k in range(NK):
            for j in range(J):
                for c in range(C):
                    for a in range(NA):
                        wks[a * J + j, k * C * J + c * J + j] = pinv[a, c] * sg[a, k]

        b0 = (pinv * Y[:, 0:1]).sum(axis=0)  # [3]
        bout = np.repeat(b0[:, None], J, axis=1).reshape(C * J, 1).astype(np.float32)

        rb = np.ascontiguousarray(raw[b].reshape(C, HW), np.float32)
        rh = rb.astype(np.float16)
        rl = (rb - rh.astype(np.float32)).astype(np.float16)
        in_maps.append(
            {
                "rawh": rh,
                "rawl": rl,
                "par": par,
                "wfh": wfh,
                "wf2": wf2,
                "wks": wks,
                "bout": bout,
            }
        )
    return in_maps


def kernel(raw, ys, A):
    raw = np.asarray(raw, np.float32)
    ys = np.asarray(ys, np.float32)
    A = np.asarray(A, np.float32)
    if "nc" not in _NC_CACHE:
        _NC_CACHE["nc"] = _build_nc()
    nc = _NC_CACHE["nc"]
    in_maps = _host_params(raw, ys, A)
    res = run_bass_kernel_spmd(nc, in_maps, core_ids=list(range(B)))
    out = np.stack([res.results[b]["out"].reshape(C, H, W) for b in range(B)])
    return out.astype(np.float32)



# revision 5
# speedup vs baseline: 1.2571x; 1.2571x over previous
"""Trainium2 Bass kernel for nn_AxisSimplestSpline (center-anchored ramp basis).

Math (per batch b, axis a), with g = (f - mins)/dx in [0,17):
  est_a(g) = V8 + sum_{k=8..16} a_k * relu(g-k) + sum_{k=1..8} c_k * min(g-k, 0)
  out[c]   = sum_a pinv[a,c] * est_a  (+ bias, folded into the drain)

Anchoring the ramp basis at g=8 keeps every feature bounded by ~8 (the
telescoped relu-from-0 form has features up to 16 whose fp16 rounding is
amplified by cancellation).  DVE features are additionally centered
(ψ = ramp - midpoint, folded into bias) so fp16 write rounding halves again.
Features are computed from the fp32 PSUM f (not from a quantized fp16 g),
so there is no coherent slope-amplified quantization of g.

Engine split per [128,1024] tile:
  PE : 2 input MMs (raw hi/lo vs wf hi/lo, same stacked rhs trick as before)
       + 17 ramp MMs, all fp16, out accumulated in PSUM      (~8.1 us)
  ACT: G32 = f*inv_dx - m (fp32), 5 small uncentered features directly
       from PSUM (Relu with per-partition bias, scale=+-1), drain+bias
  DVE: 12 centered features from G32 via dual-op tensor_scalar (2x_2P)
Emission is software-pipelined (next tile's input MMs and the previous
tile's drain are interleaved) so PE streams back-to-back and stays HAM-warm.
Numpy-simulated rel err 1.9e-3 (tolerance 2e-2).
"""

import sys

sys.path.insert(0, "/opt/trn_rl_repo")

import numpy as np

import concourse.bacc as bacc
import concourse.mybir as mybir
import concourse.tile as tile
from concourse.bass_utils import run_bass_kernel_spmd

F32 = mybir.dt.float32
F16 = mybir.dt.float16
EPS = 1e-4
B, C, H, W = 8, 3, 1024, 1024
HW = H * W
NA, K = 8, 16
J = 16
NJ = HW // J
FREE = 1024
NSUP = NJ // FREE
NCH = FREE // 512

# feature order: (side, k, engine); ACT first, then DVE production order
FEATS = (
    [("L", 1, "ACT"), ("L", 2, "ACT"), ("L", 3, "ACT"),
     ("R", 15, "ACT"), ("R", 16, "ACT")]
    + [("R", k, "DVE") for k in range(8, 15)]
    + [("L", k, "DVE") for k in range(4, 9)]
)
NF = len(FEATS)  # 17

_NC_CACHE = {}


def _build_nc():
    nc = bacc.Bacc(None, target_bir_lowering=False, debug=False)
    rawh_t = nc.dram_tensor("rawh", [C, HW], F16, kind="ExternalInput")
    rawl_t = nc.dram_tensor("rawl", [C, HW], F16, kind="ExternalInput")
    # par cols: 0 = -m (G32 bias); 1..3 = m+k for ACT-L k=1,2,3 (scale=-1);
    # 4..5 = -m-k for ACT-R k=15,16 (scale=+1)
    par_t = nc.dram_tensor("par", [128, 6], F32, kind="ExternalInput")
    wfh_t = nc.dram_tensor("wfh", [C * J, 128], F16, kind="ExternalInput")
    wf2_t = nc.dram_tensor("wf2", [2 * C * J, 128], F16, kind="ExternalInput")
    wks_t = nc.dram_tensor("wks", [128, NF * C * J], F16, kind="ExternalInput")
    bout_t = nc.dram_tensor("bout", [C * J, 1], F32, kind="ExternalInput")
    out_t = nc.dram_tensor("out", [C, HW], F16, kind="ExternalOutput")

    Relu = mybir.ActivationFunctionType.Relu
    Ident = mybir.ActivationFunctionType.Identity
    sub = mybir.AluOpType.subtract
    mx = mybir.AluOpType.max
    mn = mybir.AluOpType.min

    with tile.TileContext(nc) as tc:
        with (
            tc.tile_pool(name="const", bufs=1) as cpool,
            tc.tile_pool(name="io", bufs=3) as iopool,
            tc.tile_pool(name="g32", bufs=2) as gpool,
            tc.tile_pool(name="ff", bufs=22) as fpool,
            tc.tile_pool(name="ob", bufs=3) as obpool,
            tc.tile_pool(name="pf", bufs=2, space="PSUM") as pfpool,
            tc.tile_pool(name="po", bufs=2, space="PSUM") as popool,
        ):
            pT = cpool.tile([128, 6], F32)
            nc.sync.dma_start(out=pT[:], in_=par_t[:])
            wfh = cpool.tile([C * J, 128], F16)
            nc.sync.dma_start(out=wfh[:], in_=wfh_t[:])
            wf2 = cpool.tile([2 * C * J, 128], F16)
            nc.sync.dma_start(out=wf2[:], in_=wf2_t[:])
            wks = cpool.tile([128, NF * C * J], F16)
            nc.sync.dma_start(out=wks[:], in_=wks_t[:])
            bout = cpool.tile([C * J, 1], F32)
            nc.sync.dma_start(out=bout[:], in_=bout_t[:])

            rawh_v = rawh_t.ap().rearrange("c (j n) -> (c j) n", j=J)
            rawl_v = rawl_t.ap().rearrange("c (j n) -> (c j) n", j=J)
            out_v = out_t.ap().rearrange("c (j n) -> (c j) n", j=J)

            rhs = [None] * NSUP
            fps = [None] * NSUP
            ops = [None] * NSUP

            def drain(n):
                ob = obpool.tile([C * J, FREE], F16, tag="ob")
                nc.scalar.activation(ob[:], ops[n][:], Ident, bias=bout[:, 0:1], scale=1.0)
                n0 = n * FREE
                nc.sync.dma_start(out=out_v[:, n0 : n0 + FREE], in_=ob[:])

            def load_and_project(n):
                n0 = n * FREE
                r = iopool.tile([2 * C * J, FREE], F16, tag="rhs")
                nc.sync.dma_start(out=r[: C * J], in_=rawh_v[:, n0 : n0 + FREE])
                nc.sync.dma_start(out=r[C * J :], in_=rawl_v[:, n0 : n0 + FREE])
                f = pfpool.tile([128, FREE], F32, tag="fps")
                for h in range(NCH):
                    sl = slice(h * 512, (h + 1) * 512)
                    nc.tensor.matmul(f[:, sl], wfh[:], r[: C * J, sl], start=True, stop=False)
                    nc.tensor.matmul(f[:, sl], wf2[:], r[:, sl], start=False, stop=True)
                rhs[n], fps[n] = r, f

            load_and_project(0)

            for n in range(NSUP):
                if n + 1 < NSUP:
                    load_and_project(n + 1)

                f = fps[n]
                # ACT: G32 and its 5 uncentered features straight from PSUM
                g32 = gpool.tile([128, FREE], F32, tag="g32")
                nc.scalar.activation(g32[:], f[:], Ident, bias=pT[:, 0:1], scale=1.0)
                feats = []
                for i, (side, k, eng) in enumerate(FEATS):
                    if eng != "ACT":
                        feats.append(None)
                        continue
                    ft = fpool.tile([128, FREE], F16, tag="F")
                    if side == "L":  # max(k-g,0) = Relu(-f + (m+k))
                        nc.scalar.activation(ft[:], f[:], Relu, bias=pT[:, k : k + 1], scale=-1.0)
                    else:  # Relu(f - m - k), k=15,16 -> cols 4,5
                        nc.scalar.activation(ft[:], f[:], Relu, bias=pT[:, k - 11 : k - 10], scale=1.0)
                    feats.append(ft)

                # previous tile's drain on ACT (after this tile's ACT features)
                if n >= 1:
                    drain(n - 1)

                # DVE: centered features from G32
                for i, (side, k, eng) in enumerate(FEATS):
                    if eng != "DVE":
                        continue
                    ft = fpool.tile([128, FREE], F16, tag="F")
                    if side == "R":
                        mk = (17.0 - k) / 2.0
                        nc.vector.tensor_scalar(
                            out=ft[:], in0=g32[:], scalar1=float(k + mk),
                            scalar2=float(-mk), op0=sub, op1=mx,
                        )
                    else:
                        mk = k / 2.0
                        nc.vector.tensor_scalar(
                            out=ft[:], in0=g32[:], scalar1=float(k - mk),
                            scalar2=float(mk), op0=sub, op1=mn,
                        )
                    feats[i] = ft

                o = popool.tile([C * J, FREE], F32, tag="ops")
                for h in range(NCH):
                    sl = slice(h * 512, (h + 1) * 512)
                    for i in range(NF):
                        nc.tensor.matmul(
                            o[:, sl],
                            wks[:, i * C * J : (i + 1) * C * J],
                            feats[i][:, sl],
                            start=(i == 0),
                            stop=(i == NF - 1),
                        )
                ops[n] = o

            drain(NSUP - 1)
    nc.compile()
    return nc


def _host_params(raw, ys, A):
    in_maps = []
    jr = lambda x: np.repeat(x, J)
    for b in range(B):
        Ab = A[b].astype(np.float32)
        mins = np.minimum(Ab, 0).sum(axis=0)
        maxs = np.maximum(Ab, 0).sum(axis=0)
        pinv = np.linalg.pinv(Ab).astype(np.float32)  # [8, 3]
        span = (maxs + np.float32(EPS) - mins).astype(np.float32)
        dx = (span / np.float32(K + 1)).astype(np.float32)
        inv_dx = (np.float32(1.0) / dx).astype(np.float32)
        Y = np.concatenate(
            [mins[:, None], ys[b].astype(np.float32), maxs[:, None]], axis=1
        )  # [8, 18]
        s = np.diff(Y, axis=1).astype(np.float32)  # [8, 17]

        a = np.zeros((NA, 17), np.float32)
        c = np.zeros((NA, 17), np.float32)
        a[:, 8] = s[:, 8]
        for k in range(9, 17):
            a[:, k] = s[:, k] - s[:, k - 1]
        c[:, 8] = s[:, 7]
        for k in range(1, 8):
            c[:, k] = s[:, k - 1] - s[:, k]

        m = (mins * inv_dx).astype(np.float32)
        par = np.zeros((128, 6), np.float32)
        par[:, 0] = jr(-m)
        for k in (1, 2, 3):
            par[:, k] = jr(m + k)
        for k in (15, 16):
            par[:, k - 11] = jr(-m - k)

        # input weights with inv_dx folded, hi/lo split
        wf = (Ab * inv_dx[None, :]).astype(np.float32)  # [3, 8]
        wfm = np.zeros((C * J, 128), np.float32)
        for jj in range(J):
            for cc in range(C):
                for aa in range(NA):
                    wfm[cc * J + jj, aa * J + jj] = wf[cc, aa]
        wfh = wfm.astype(np.float16)
        wfl = (wfm - wfh.astype(np.float32)).astype(np.float16)
        wf2 = np.concatenate([wfl, wfh], axis=0)  # rows 0:48 hit raw-hi, 48:96 raw-lo

        bias = Y[:, 8].astype(np.float32).copy()  # V8
        wco = np.zeros((NA, NF), np.float32)
        for i, (side, k, eng) in enumerate(FEATS):
            if side == "R":
                wco[:, i] = a[:, k]
                if eng == "DVE":
                    bias += a[:, k] * np.float32((17.0 - k) / 2.0)
            else:
                if eng == "ACT":
                    wco[:, i] = -c[:, k]
                else:
                    wco[:, i] = c[:, k]
                    bias -= c[:, k] * np.float32(k / 2.0)

        wks = np.zeros((128, NF * C * J), np.float16)
        for i in range(NF):
            for jj in range(J):
                for cc in range(C):
                    for aa in range(NA):
                        wks[aa * J + jj, i * C * J + cc * J + jj] = (
                            pinv[aa, cc] * wco[aa, i]
                        )

        b0 = (pinv * bias[:, None]).sum(axis=0)  # [3]
        bout = np.repeat(b0[:, None], J, axis=1).reshape(C * J, 1).astype(np.float32)

        rb = np.ascontiguousarray(raw[b].reshape(C, HW), np.float32)
        rh = rb.astype(np.float16)
        rl = (rb - rh.astype(np.float32)).astype(np.float16)
        in_maps.append(
            {
                "rawh": rh,
                "rawl": rl,
                "par": par,
                "wfh": wfh,
                "wf2": wf2,
                "wks": wks,
                "bout": bout,
            }
        )
    return in_maps


def kernel(raw, ys, A):
    raw = np.asarray(raw, np.float32)
    ys = np.asarray(ys, np.float32)
    A = np.asarray(A, np.float32)
    if "nc" not in _NC_CACHE:
        _NC_CACHE["nc"] = _build_nc()
    nc = _NC_CACHE["nc"]
    in_maps = _host_params(raw, ys, A)
    res = run_bass_kernel_spmd(nc, in_maps, core_ids=list(range(B)))
    out = np.stack(
        [res.results[b]["out"].astype(np.float32).reshape(C, H, W) for b in range(B)]
    )
    return out.astype(np.float32)
